# revision 21
# baseline (speedup 1.0000x reference)
"""GaussianVoxelizer Trainium2 kernel (8 NeuronCores).

Strategy: the 200x200x16 output grid is sharded over 8 cores as 25-row
I-slabs.  On the host, each gaussian is routed (with duplication) to the
I-slabs and 7-wide J-bins its 3-sigma box overlaps.  Per (slab, J-bin)
the device evaluates, for a batch of <=128 gaussians, the Mahalanobis
quadratic over the bin's 25x7x16 voxel window with ONE fp32 matmul
(one-hot axis tables + centered cross monomials as the stationary
basis; box masks enter as additive barriers, opacity as -2*ln(op)),
takes exp on the scalar engine, and contracts gaussians against the
18 feature channels (17 feats + 1 density row) with a bf16 matmul
accumulated in PSUM.  A second device phase repartitions the raw sums
to 128 partitions, adds the (constant, separable) "empty gaussian"
density, and performs the normalization divide.  Host only routes
inputs and reassembles the sharded outputs.
"""

import os
import sys
import numpy as np

for _p in ("/opt/trn_rl_repo", "/opt/pypackages"):
    if os.path.isdir(_p) and _p not in sys.path:
        sys.path.append(_p)

import ml_dtypes

# ---------------------------------------------------------------- constants
VOL_LO = np.array([-40.0, -40.0, -1.0], np.float32)
VOL_HI = np.array([40.0, 40.0, 5.4], np.float32)
VOXEL = np.float32(0.4)
GRID = (200, 200, 16)
NCORES = 8
SLAB = 25            # I-rows per core
BJ = 7               # J-bin width
NBINS = (GRID[1] + BJ - 1) // BJ       # 29 (last bin is 4 wide)
WIN = SLAB * BJ * GRID[2]              # 2800 voxels per window
K1 = SLAB + BJ + GRID[2] + 3           # 51 basis rows for the maha matmul
CI, CJ, CK = 12.0, 3.0, 7.5            # fixed centering of cross monomials
BARRIER = 4000.0
NCH = 18                               # 17 feature channels + density row
CHUNK = 512
VPP = (SLAB * GRID[1] * GRID[2]) // 128  # 625 voxels per partition in phase 2


def _chunks(total, step):
    return [(c, min(step, total - c)) for c in range(0, total, step)]


# ---------------------------------------------------------------- host prep
def _quat_to_rotmat(q):
    q = q / np.linalg.norm(q, axis=-1, keepdims=True)
    w, x, y, z = q[..., 0], q[..., 1], q[..., 2], q[..., 3]
    R = np.stack(
        [
            1 - 2 * (y * y + z * z), 2 * (x * y - w * z), 2 * (x * z + w * y),
            2 * (x * y + w * z), 1 - 2 * (x * x + z * z), 2 * (y * z - w * x),
            2 * (x * z - w * y), 2 * (y * z + w * x), 1 - 2 * (x * x + y * y),
        ],
        axis=-1,
    ).astype(np.float32)
    return R.reshape(q.shape[:-1] + (3, 3))


def _host_prep(means3d, opacities, features, scales, rotations):
    """Mirror of the reference's per-gaussian math (numpy f32) + routing.

    Returns per-core lhs tables and the static (bin -> nbatch) schedule.
    """
    N = means3d.shape[0]
    lo, hi = VOL_LO, VOL_HI

    R = _quat_to_rotmat(rotations.astype(np.float32))
    Lm = R * scales.astype(np.float32)[:, None, :]
    cov = (Lm @ np.swapaxes(Lm, -1, -2)).astype(np.float32)
    cov_inv = np.linalg.inv(cov).astype(np.float32)
    sig = np.sqrt(np.diagonal(cov, axis1=-2, axis2=-1)).astype(np.float32)

    b_lo = np.clip(means3d - np.float32(3.0) * sig, lo, hi).astype(np.float32)
    b_hi = np.clip(means3d + np.float32(3.0) * sig, lo, hi).astype(np.float32)
    valid = (((b_lo > lo) | (b_hi > lo)) & ((b_lo < hi) | (b_hi < hi))).all(-1)
    i_lo = ((b_lo - lo) / VOXEL).astype(np.int32)
    i_hi = ((b_hi - lo) / VOXEL).astype(np.int32)

    # inclusive, grid-clamped box
    box_lo = np.maximum(i_lo, 0)
    box_hi = np.minimum(i_hi, np.array(GRID, np.int32) - 1)
    valid &= (box_lo <= box_hi).all(-1)

    Ct = (VOXEL * VOXEL) * cov_inv          # quadratic form in voxel coords
    m_t = (means3d - lo) / VOXEL - np.float32(0.5)  # voxel-coord mean
    op = np.maximum(opacities[:, 0].astype(np.float32), np.float32(1e-30))
    lnop = np.log(op).astype(np.float32)

    # ---- routing: entries[(core, bin)] = list of gaussian ids
    entries = [[[] for _ in range(NBINS)] for _ in range(NCORES)]
    idxs = np.nonzero(valid)[0]
    for g in idxs:
        s0 = box_lo[g, 0] // SLAB
        s1 = box_hi[g, 0] // SLAB
        b0 = box_lo[g, 1] // BJ
        b1 = box_hi[g, 1] // BJ
        for s in range(s0, s1 + 1):
            for b in range(b0, b1 + 1):
                entries[s][b].append(g)

    # static schedule: nbatch per bin = max over cores
    nbatch = [
        max(1, max((len(entries[s][b]) + 127) // 128 for s in range(NCORES)))
        for b in range(NBINS)
    ]
    nbb = sum(nbatch)

    il = np.arange(SLAB, dtype=np.float32)
    jl = np.arange(BJ, dtype=np.float32)
    kk = np.arange(GRID[2], dtype=np.float32)

    lhs1 = np.zeros((NCORES, nbb, K1, 128), np.float32)
    lhs1[:, :, 0:SLAB, :] = BARRIER          # pad gaussians -> dens 0
    lhs2 = np.zeros((NCORES, nbb, 128, NCH), np.float32)

    for s in range(NCORES):
        bb = 0
        for b in range(NBINS):
            glist = entries[s][b]
            for t in range(nbatch[b]):
                part = glist[t * 128:(t + 1) * 128]
                if part:
                    gs = np.asarray(part)
                    n = len(gs)
                    aI = m_t[gs, 0] - np.float32(SLAB * s)
                    aJ = m_t[gs, 1] - np.float32(BJ * b)
                    ak = m_t[gs, 2]
                    c00 = Ct[gs, 0, 0]; c11 = Ct[gs, 1, 1]; c22 = Ct[gs, 2, 2]
                    c01 = Ct[gs, 0, 1]; c02 = Ct[gs, 0, 2]; c12 = Ct[gs, 1, 2]

                    LI = 2 * c01 * (CJ - aJ) + 2 * c02 * (CK - ak)
                    LJ = 2 * c01 * (CI - aI) + 2 * c12 * (CK - ak)
                    Lk = 2 * c02 * (CI - aI) + 2 * c12 * (CJ - aJ)
                    CONST = (
                        2 * (c01 * (CI - aI) * (CJ - aJ)
                             + c02 * (CI - aI) * (CK - ak)
                             + c12 * (CJ - aJ) * (CK - ak))
                        - 2 * lnop[gs]
                    )

                    tabI = (c00[:, None] * (il[None] - aI[:, None]) ** 2
                            + LI[:, None] * (il[None] - CI))
                    tabJ = (c11[:, None] * (jl[None] - aJ[:, None]) ** 2
                            + LJ[:, None] * (jl[None] - CJ))
                    tabk = (c22[:, None] * (kk[None] - ak[:, None]) ** 2
                            + Lk[:, None] * (kk[None] - CK) + CONST[:, None])

                    # box-mask barriers (global coords)
                    gi = SLAB * s + np.arange(SLAB)[None]
                    gj = BJ * b + np.arange(BJ)[None]
                    gk = np.arange(GRID[2])[None]
                    tabI = np.where((gi < box_lo[gs, 0:1]) | (gi > box_hi[gs, 0:1]),
                                    np.float32(BARRIER), tabI)
                    tabJ = np.where((gj < box_lo[gs, 1:2]) | (gj > box_hi[gs, 1:2]),
                                    np.float32(BARRIER), tabJ)
                    tabk = np.where((gk < box_lo[gs, 2:3]) | (gk > box_hi[gs, 2:3]),
                                    np.float32(BARRIER), tabk)

                    col = np.concatenate(
                        [tabI, tabJ, tabk,
                         2 * c01[:, None], 2 * c02[:, None], 2 * c12[:, None]],
                        axis=1,
                    )  # [n, K1]
                    lhs1[s, bb, :, :n] = col.T
                    lhs2[s, bb, :n, 0:17] = features[gs].astype(np.float32)
                    lhs2[s, bb, :n, 17] = 1.0
                bb += 1
    return lhs1, lhs2, nbatch


def _basis_rhs():
    """[K1, WIN] shared basis: one-hots + centered cross monomials.

    Window voxel order is J-major: v = jl*400 + il*16 + k.
    """
    il, jl, kk = np.meshgrid(
        np.arange(SLAB), np.arange(BJ), np.arange(GRID[2]), indexing="ij"
    )
    v = jl * (SLAB * GRID[2]) + il * GRID[2] + kk
    rhs = np.zeros((K1, WIN), np.float32)
    fil, fjl, fkk = (x.reshape(-1) for x in (il, jl, kk))
    fv = v.reshape(-1)
    for r in range(SLAB):
        rhs[r, fv[fil == r]] = 1.0
    for r in range(BJ):
        rhs[SLAB + r, fv[fjl == r]] = 1.0
    for r in range(GRID[2]):
        rhs[SLAB + BJ + r, fv[fkk == r]] = 1.0
    rhs[K1 - 3, fv] = (fil - CI) * (fjl - CJ)
    rhs[K1 - 2, fv] = (fil - CI) * (fkk - CK)
    rhs[K1 - 1, fv] = (fjl - CJ) * (fkk - CK)
    return rhs


def _dens_e_vprime():
    """Empty-gaussian density for the full grid, per-core, v' = j*400+i*16+k."""
    H, W, D = GRID
    ii, jj, kk = np.meshgrid(
        np.arange(H, dtype=np.float32),
        np.arange(W, dtype=np.float32),
        np.arange(D, dtype=np.float32),
        indexing="ij",
    )
    gc = np.stack([ii, jj, kk], -1) + np.float32(0.5)
    gc = gc * VOXEL + VOL_LO
    e_mean = (VOL_LO + VOL_HI) / 2
    e_rng = VOL_HI - VOL_LO
    de = (gc - e_mean) / e_rng
    maha = (de * de).sum(-1)
    dens_e = np.exp(np.float32(-0.5) * maha).astype(np.float32)  # [200,200,16]
    out = []
    for s in range(NCORES):
        slab = dens_e[SLAB * s:SLAB * (s + 1)]          # [25,200,16]
        out.append(slab.transpose(1, 0, 2).reshape(128, VPP).copy())
    return out


# ------------------------------------------------------------- golden model
def _golden_core(lhs1, lhs2, nbatch, rhs1, dens_e_vp, es):
    """Numpy simulation of the device program for one core.

    lhs1: [nbb, 2*K1, 128] fp16 hi/lo stacked; rhs1: [2*K1, WIN] fp16.
    """
    scratch = np.zeros((NCH, SLAB * GRID[1] * GRID[2]), np.float32)
    bb = 0
    for b in range(NBINS):
        jw = min(BJ, GRID[1] - BJ * b)
        acc = np.zeros((NCH, WIN), np.float32)
        for t in range(nbatch[b]):
            maha = lhs1[bb].T.astype(np.float32) @ rhs1.astype(np.float32)
            dens = np.exp(-0.5 * maha).astype(ml_dtypes.bfloat16)
            acc += lhs2[bb].astype(ml_dtypes.bfloat16).T.astype(np.float32) @ \
                dens.astype(np.float32)
            bb += 1
        w = jw * SLAB * GRID[2]
        scratch[:, b * BJ * SLAB * GRID[2]:][:, :w] = acc[:, :w]
    fin = scratch.reshape(NCH, 128, VPP).transpose(1, 0, 2).copy()
    d_tot = fin[:, 17] + dens_e_vp
    r = 1.0 / d_tot
    feats = fin[:, 0:17] * r[:, None, :]
    ch17 = dens_e_vp * es * r
    out_feats = np.concatenate([feats, ch17[:, None, :]], 1)  # [128,18,625]
    return out_feats.transpose(1, 0, 2).reshape(NCH, -1), d_tot.reshape(-1)


# ------------------------------------------------------------ device kernel
_CACHE = {}
LAST_RESULT = None  # BassKernelResults of the most recent device run


def _build_program(nbatch):
    import concourse.mybir as mybir
    from concourse import bacc, tile

    f32 = mybir.dt.float32
    bf16 = mybir.dt.bfloat16
    f16 = mybir.dt.float16
    K1HL = 2 * K1
    nbb = sum(nbatch)
    NV = SLAB * GRID[1] * GRID[2]  # 80000

    nc = bacc.Bacc("TRN2", target_bir_lowering=False, debug=True)
    lhs1_d = nc.dram_tensor("lhs1", [nbb, K1HL, 128], f16, kind="ExternalInput")
    lhs2_d = nc.dram_tensor("lhs2", [nbb, 128, NCH], bf16, kind="ExternalInput")
    rhs1_d = nc.dram_tensor("rhs1", [K1HL, WIN], f16, kind="ExternalInput")
    dens_e_d = nc.dram_tensor("dens_e", [128, VPP], f32, kind="ExternalInput")
    es_d = nc.dram_tensor("es", [128, 1], f32, kind="ExternalInput")
    feats_d = nc.dram_tensor("out_feats", [NCH, NV], f32, kind="ExternalOutput")
    dens_d = nc.dram_tensor("out_dens", [NV], f32, kind="ExternalOutput")
    scratch_d = nc.dram_tensor("scratch", [NCH, NV], f32)

    with tile.TileContext(nc) as tc:
        HALF = WIN // 2  # 1400
        with (
            tc.tile_pool(name="const", bufs=1) as constp,
            tc.tile_pool(name="w1", bufs=3) as w1p,
            tc.tile_pool(name="w2", bufs=3) as w2p,
            tc.tile_pool(name="dens", bufs=4) as densp,
            tc.tile_pool(name="stage", bufs=2) as stagep,
            tc.tile_pool(name="pm", bufs=2, space="PSUM") as pmp,
            tc.tile_pool(name="po", bufs=1, space="PSUM") as pop,
        ):
            rhs1_t = constp.tile([K1HL, WIN], f16)
            nc.sync.dma_start(out=rhs1_t[:], in_=rhs1_d[:])

            bb = 0
            for b in range(NBINS):
                jw = min(BJ, GRID[1] - BJ * b)
                w = jw * SLAB * GRID[2]
                stage_t = stagep.tile([NCH, WIN], f32, tag="stage")
                lhs1_ts = []
                lhs2_ts = []
                for t in range(nbatch[b]):
                    l1 = w1p.tile([K1HL, 128], f16, tag="l1")
                    nc.gpsimd.dma_start(out=l1[:], in_=lhs1_d[bb + t])
                    l2 = w2p.tile([128, NCH], bf16, tag="l2")
                    nc.gpsimd.dma_start(out=l2[:], in_=lhs2_d[bb + t])
                    lhs1_ts.append(l1)
                    lhs2_ts.append(l2)
                for h0 in (0, HALF):
                    po_t = pop.tile([NCH, HALF], f32, tag="po")
                    for c0, cw in ((0, 1024), (1024, HALF - 1024)):
                        for t in range(nbatch[b]):
                            pm_t = pmp.tile([128, 1024], f32, tag="pm")
                            for m0 in range(0, cw, CHUNK):
                                mw = min(CHUNK, cw - m0)
                                nc.tensor.matmul(
                                    out=pm_t[:, m0:m0 + mw],
                                    lhsT=lhs1_ts[t][:],
                                    rhs=rhs1_t[
                                        :, h0 + c0 + m0:h0 + c0 + m0 + mw
                                    ],
                                    start=True,
                                    stop=True,
                                )
                            d_t = densp.tile([128, 1024], bf16, tag="d")
                            nc.scalar.activation(
                                out=d_t[:, :cw],
                                in_=pm_t[:, :cw],
                                func=mybir.ActivationFunctionType.Exp,
                                scale=-0.5,
                            )
                            for m0 in range(0, cw, CHUNK):
                                mw = min(CHUNK, cw - m0)
                                nc.tensor.matmul(
                                    out=po_t[:, c0 + m0:c0 + m0 + mw],
                                    lhsT=lhs2_ts[t][:],
                                    rhs=d_t[:, m0:m0 + mw],
                                    start=(t == 0),
                                    stop=(t == nbatch[b] - 1),
                                )
                    nc.vector.tensor_copy(
                        out=stage_t[:, h0:h0 + HALF], in_=po_t[:]
                    )
                bb += nbatch[b]
                off = b * BJ * SLAB * GRID[2]
                nc.sync.dma_start(
                    out=scratch_d[:, off:off + w], in_=stage_t[:, :w]
                )

        # ---- phase 2: repartition + empty term + normalize ----
        with (
            tc.tile_pool(name="fin", bufs=1) as finp,
            tc.tile_pool(name="aux", bufs=1) as auxp,
        ):
            fin_t = finp.tile([128, NCH, VPP], f32)
            nc.sync.dma_start(
                out=fin_t[:],
                in_=scratch_d[:].rearrange("c (p r) -> p c r", p=128),
            )
            de_t = auxp.tile([128, VPP], f32)
            nc.sync.dma_start(out=de_t[:], in_=dens_e_d[:])
            es_t = auxp.tile([128, 1], f32)
            nc.sync.dma_start(out=es_t[:], in_=es_d[:])

            dtot_t = auxp.tile([128, VPP], f32)
            nc.vector.tensor_add(out=dtot_t[:], in0=fin_t[:, 17], in1=de_t[:])
            r_t = auxp.tile([128, VPP], f32)
            nc.vector.reciprocal(out=r_t[:], in_=dtot_t[:])
            for c in range(17):
                eng = nc.gpsimd if c % 2 else nc.vector
                eng.tensor_mul(out=fin_t[:, c], in0=fin_t[:, c], in1=r_t[:])
            # ch17 = es * dens_e * r
            nc.vector.tensor_mul(out=de_t[:], in0=de_t[:], in1=r_t[:])
            nc.vector.tensor_scalar_mul(
                out=fin_t[:, 17], in0=de_t[:], scalar1=es_t[:]
            )
            nc.sync.dma_start(
                out=feats_d[:].rearrange("c (p r) -> p c r", p=128),
                in_=fin_t[:],
            )
            nc.sync.dma_start(
                out=dens_d[:].rearrange("(p r) -> p r", p=128), in_=dtot_t[:]
            )
    if not nc.is_finalized():
        nc.finalize()
    return nc


def _hilo_fp16(lhs1):
    """Split fp32 tables into stacked fp16 hi/lo rows (one bf16-rate matmul)."""
    hi = lhs1.astype(np.float16)
    lo = (lhs1 - hi.astype(np.float32)).astype(np.float16)
    return np.concatenate([hi, lo], axis=-2)  # [..., 2*K1, 128]


def kernel(means3d, opacities, features, scales, rotations, empty_scalar):
    lhs1, lhs2, nbatch = _host_prep(
        means3d, opacities, features, scales, rotations
    )
    rhs1 = _basis_rhs()
    dens_e_cores = _dens_e_vprime()
    es = np.float32(np.asarray(empty_scalar).reshape(-1)[0])

    H, W, D = GRID
    rhs1_hl = np.concatenate(
        [rhs1.astype(np.float16), rhs1.astype(np.float16)], axis=0
    )  # [2*K1, WIN]

    if os.environ.get("KERNEL_GOLDEN"):
        feats_parts, dens_parts = [], []
        for s in range(NCORES):
            f, d = _golden_core(
                _hilo_fp16(lhs1[s]), lhs2[s].astype(ml_dtypes.bfloat16),
                nbatch, rhs1_hl, dens_e_cores[s], es,
            )
            feats_parts.append(f)
            dens_parts.append(d)
    else:
        from concourse.bass_utils import run_bass_kernel_spmd

        key = tuple(nbatch)
        if key not in _CACHE:
            _CACHE[key] = _build_program(nbatch)
        nc = _CACHE[key]
        in_maps = [
            {
                "lhs1": np.ascontiguousarray(_hilo_fp16(lhs1[s])),
                "lhs2": np.ascontiguousarray(lhs2[s]).astype(ml_dtypes.bfloat16),
                "rhs1": rhs1_hl,
                "dens_e": dens_e_cores[s],
                "es": np.full((128, 1), es, np.float32),
            }
            for s in range(NCORES)
        ]
        res = run_bass_kernel_spmd(
            nc,
            in_maps,
            list(range(NCORES)),
            trace=bool(os.environ.get("KERNEL_TRACE")),
        )
        global LAST_RESULT
        LAST_RESULT = res
        feats_parts = [res.results[s]["out_feats"] for s in range(NCORES)]
        dens_parts = [res.results[s]["out_dens"] for s in range(NCORES)]

    feats = np.concatenate(
        [
            f.reshape(NCH, W, SLAB, D).transpose(2, 1, 3, 0)
            for f in feats_parts
        ],
        axis=0,
    )  # [200,200,16,18]
    dens = np.concatenate(
        [d.reshape(W, SLAB, D).transpose(1, 0, 2) for d in dens_parts], axis=0
    )[..., None]  # [200,200,16,1]
    return dens.astype(np.float32), feats.astype(np.float32)


# revision 24
# speedup vs baseline: 1.2665x; 1.2665x over previous
"""GaussianVoxelizer Trainium2 kernel (8 NeuronCores).

Strategy: the 200x200x16 output grid is sharded over 8 cores as 25-row
I-slabs.  On the host, each gaussian is routed (with duplication) to the
I-slabs and 7-wide J-bins its 3-sigma box overlaps.  Per (slab, J-bin)
the device evaluates, for a batch of <=128 gaussians, the Mahalanobis
quadratic over the bin's 25x7x16 voxel window with ONE fp32 matmul
(one-hot axis tables + centered cross monomials as the stationary
basis; box masks enter as additive barriers, opacity as -2*ln(op)),
takes exp on the scalar engine, and contracts gaussians against the
18 feature channels (17 feats + 1 density row) with a bf16 matmul
accumulated in PSUM.  A second device phase repartitions the raw sums
to 128 partitions, adds the (constant, separable) "empty gaussian"
density, and performs the normalization divide.  Host only routes
inputs and reassembles the sharded outputs.
"""

import os
import sys
import numpy as np

for _p in ("/opt/trn_rl_repo", "/opt/pypackages"):
    if os.path.isdir(_p) and _p not in sys.path:
        sys.path.append(_p)

import ml_dtypes

# ---------------------------------------------------------------- constants
VOL_LO = np.array([-40.0, -40.0, -1.0], np.float32)
VOL_HI = np.array([40.0, 40.0, 5.4], np.float32)
VOXEL = np.float32(0.4)
GRID = (200, 200, 16)
NCORES = 8
SLAB = 25            # I-rows per core
BJ = 7               # J-bin width
NBINS = (GRID[1] + BJ - 1) // BJ       # 29 (last bin is 4 wide)
WIN = SLAB * BJ * GRID[2]              # 2800 voxels per window
K1 = SLAB + BJ + GRID[2] + 3           # 51 basis rows for the maha matmul
CI, CJ, CK = 12.0, 3.0, 7.5            # fixed centering of cross monomials
BARRIER = 4000.0
NCH = 18                               # 17 feature channels + density row
CHUNK = 512
VPP = (SLAB * GRID[1] * GRID[2]) // 128  # 625 voxels per partition in phase 2


def _chunks(total, step):
    return [(c, min(step, total - c)) for c in range(0, total, step)]


# ---------------------------------------------------------------- host prep
def _quat_to_rotmat(q):
    q = q / np.linalg.norm(q, axis=-1, keepdims=True)
    w, x, y, z = q[..., 0], q[..., 1], q[..., 2], q[..., 3]
    R = np.stack(
        [
            1 - 2 * (y * y + z * z), 2 * (x * y - w * z), 2 * (x * z + w * y),
            2 * (x * y + w * z), 1 - 2 * (x * x + z * z), 2 * (y * z - w * x),
            2 * (x * z - w * y), 2 * (y * z + w * x), 1 - 2 * (x * x + y * y),
        ],
        axis=-1,
    ).astype(np.float32)
    return R.reshape(q.shape[:-1] + (3, 3))


def _host_prep(means3d, opacities, features, scales, rotations):
    """Mirror of the reference's per-gaussian math (numpy f32) + routing.

    Returns per-core lhs tables and the static (bin -> nbatch) schedule.
    """
    N = means3d.shape[0]
    lo, hi = VOL_LO, VOL_HI

    R = _quat_to_rotmat(rotations.astype(np.float32))
    Lm = R * scales.astype(np.float32)[:, None, :]
    cov = (Lm @ np.swapaxes(Lm, -1, -2)).astype(np.float32)
    cov_inv = np.linalg.inv(cov).astype(np.float32)
    sig = np.sqrt(np.diagonal(cov, axis1=-2, axis2=-1)).astype(np.float32)

    b_lo = np.clip(means3d - np.float32(3.0) * sig, lo, hi).astype(np.float32)
    b_hi = np.clip(means3d + np.float32(3.0) * sig, lo, hi).astype(np.float32)
    valid = (((b_lo > lo) | (b_hi > lo)) & ((b_lo < hi) | (b_hi < hi))).all(-1)
    i_lo = ((b_lo - lo) / VOXEL).astype(np.int32)
    i_hi = ((b_hi - lo) / VOXEL).astype(np.int32)

    # inclusive, grid-clamped box
    box_lo = np.maximum(i_lo, 0)
    box_hi = np.minimum(i_hi, np.array(GRID, np.int32) - 1)
    valid &= (box_lo <= box_hi).all(-1)

    Ct = (VOXEL * VOXEL) * cov_inv          # quadratic form in voxel coords
    m_t = (means3d - lo) / VOXEL - np.float32(0.5)  # voxel-coord mean
    op = np.maximum(opacities[:, 0].astype(np.float32), np.float32(1e-30))
    lnop = np.log(op).astype(np.float32)

    # ---- routing: entries[(core, bin)] = list of gaussian ids
    entries = [[[] for _ in range(NBINS)] for _ in range(NCORES)]
    idxs = np.nonzero(valid)[0]
    for g in idxs:
        s0 = box_lo[g, 0] // SLAB
        s1 = box_hi[g, 0] // SLAB
        b0 = box_lo[g, 1] // BJ
        b1 = box_hi[g, 1] // BJ
        for s in range(s0, s1 + 1):
            for b in range(b0, b1 + 1):
                entries[s][b].append(g)

    # static schedule: nbatch per bin = max over cores
    nbatch = [
        max(1, max((len(entries[s][b]) + 127) // 128 for s in range(NCORES)))
        for b in range(NBINS)
    ]
    nbb = sum(nbatch)

    il = np.arange(SLAB, dtype=np.float32)
    jl = np.arange(BJ, dtype=np.float32)
    kk = np.arange(GRID[2], dtype=np.float32)

    lhs1 = np.zeros((NCORES, nbb, K1, 128), np.float32)
    lhs1[:, :, 0:SLAB, :] = BARRIER          # pad gaussians -> dens 0
    lhs2 = np.zeros((NCORES, nbb, 128, NCH), np.float32)

    for s in range(NCORES):
        bb = 0
        for b in range(NBINS):
            glist = entries[s][b]
            for t in range(nbatch[b]):
                part = glist[t * 128:(t + 1) * 128]
                if part:
                    gs = np.asarray(part)
                    n = len(gs)
                    aI = m_t[gs, 0] - np.float32(SLAB * s)
                    aJ = m_t[gs, 1] - np.float32(BJ * b)
                    ak = m_t[gs, 2]
                    c00 = Ct[gs, 0, 0]; c11 = Ct[gs, 1, 1]; c22 = Ct[gs, 2, 2]
                    c01 = Ct[gs, 0, 1]; c02 = Ct[gs, 0, 2]; c12 = Ct[gs, 1, 2]

                    LI = 2 * c01 * (CJ - aJ) + 2 * c02 * (CK - ak)
                    LJ = 2 * c01 * (CI - aI) + 2 * c12 * (CK - ak)
                    Lk = 2 * c02 * (CI - aI) + 2 * c12 * (CJ - aJ)
                    CONST = (
                        2 * (c01 * (CI - aI) * (CJ - aJ)
                             + c02 * (CI - aI) * (CK - ak)
                             + c12 * (CJ - aJ) * (CK - ak))
                        - 2 * lnop[gs]
                    )

                    tabI = (c00[:, None] * (il[None] - aI[:, None]) ** 2
                            + LI[:, None] * (il[None] - CI))
                    tabJ = (c11[:, None] * (jl[None] - aJ[:, None]) ** 2
                            + LJ[:, None] * (jl[None] - CJ))
                    tabk = (c22[:, None] * (kk[None] - ak[:, None]) ** 2
                            + Lk[:, None] * (kk[None] - CK) + CONST[:, None])

                    # box-mask barriers (global coords)
                    gi = SLAB * s + np.arange(SLAB)[None]
                    gj = BJ * b + np.arange(BJ)[None]
                    gk = np.arange(GRID[2])[None]
                    tabI = np.where((gi < box_lo[gs, 0:1]) | (gi > box_hi[gs, 0:1]),
                                    np.float32(BARRIER), tabI)
                    tabJ = np.where((gj < box_lo[gs, 1:2]) | (gj > box_hi[gs, 1:2]),
                                    np.float32(BARRIER), tabJ)
                    tabk = np.where((gk < box_lo[gs, 2:3]) | (gk > box_hi[gs, 2:3]),
                                    np.float32(BARRIER), tabk)

                    col = np.concatenate(
                        [tabI, tabJ, tabk,
                         2 * c01[:, None], 2 * c02[:, None], 2 * c12[:, None]],
                        axis=1,
                    )  # [n, K1]
                    lhs1[s, bb, :, :n] = col.T
                    lhs2[s, bb, :n, 0:17] = features[gs].astype(np.float32)
                    lhs2[s, bb, :n, 17] = 1.0
                bb += 1
    return lhs1, lhs2, nbatch


def _basis_rhs():
    """[K1, WIN] shared basis: one-hots + centered cross monomials.

    Window voxel order is J-major: v = jl*400 + il*16 + k.
    """
    il, jl, kk = np.meshgrid(
        np.arange(SLAB), np.arange(BJ), np.arange(GRID[2]), indexing="ij"
    )
    v = jl * (SLAB * GRID[2]) + il * GRID[2] + kk
    rhs = np.zeros((K1, WIN), np.float32)
    fil, fjl, fkk = (x.reshape(-1) for x in (il, jl, kk))
    fv = v.reshape(-1)
    for r in range(SLAB):
        rhs[r, fv[fil == r]] = 1.0
    for r in range(BJ):
        rhs[SLAB + r, fv[fjl == r]] = 1.0
    for r in range(GRID[2]):
        rhs[SLAB + BJ + r, fv[fkk == r]] = 1.0
    rhs[K1 - 3, fv] = (fil - CI) * (fjl - CJ)
    rhs[K1 - 2, fv] = (fil - CI) * (fkk - CK)
    rhs[K1 - 1, fv] = (fjl - CJ) * (fkk - CK)
    return rhs


def _dens_e_vprime():
    """Empty-gaussian density for the full grid, per-core, v' = j*400+i*16+k."""
    H, W, D = GRID
    ii, jj, kk = np.meshgrid(
        np.arange(H, dtype=np.float32),
        np.arange(W, dtype=np.float32),
        np.arange(D, dtype=np.float32),
        indexing="ij",
    )
    gc = np.stack([ii, jj, kk], -1) + np.float32(0.5)
    gc = gc * VOXEL + VOL_LO
    e_mean = (VOL_LO + VOL_HI) / 2
    e_rng = VOL_HI - VOL_LO
    de = (gc - e_mean) / e_rng
    maha = (de * de).sum(-1)
    dens_e = np.exp(np.float32(-0.5) * maha).astype(np.float32)  # [200,200,16]
    out = []
    for s in range(NCORES):
        slab = dens_e[SLAB * s:SLAB * (s + 1)]          # [25,200,16]
        out.append(slab.transpose(1, 0, 2).reshape(128, VPP).copy())
    return out


# ------------------------------------------------------------- golden model
def _golden_core(lhs1, lhs2, nbatch, rhs1, dens_e_vp, es):
    """Numpy simulation of the device program for one core.

    lhs1: [nbb, 2*K1, 128] fp16 hi/lo stacked; rhs1: [2*K1, WIN] fp16.
    """
    scratch = np.zeros((NCH, SLAB * GRID[1] * GRID[2]), np.float32)
    bb = 0
    for b in range(NBINS):
        jw = min(BJ, GRID[1] - BJ * b)
        acc = np.zeros((NCH, WIN), np.float32)
        for t in range(nbatch[b]):
            maha = lhs1[bb].T.astype(np.float32) @ rhs1.astype(np.float32)
            dens = np.exp(-0.5 * maha).astype(ml_dtypes.bfloat16)
            acc += lhs2[bb].astype(ml_dtypes.bfloat16).T.astype(np.float32) @ \
                dens.astype(np.float32)
            bb += 1
        w = jw * SLAB * GRID[2]
        scratch[:, b * BJ * SLAB * GRID[2]:][:, :w] = acc[:, :w]
    fin = scratch.reshape(NCH, 128, VPP).transpose(1, 0, 2).copy()
    d_tot = fin[:, 17] + dens_e_vp
    r = 1.0 / d_tot
    feats = fin[:, 0:17] * r[:, None, :]
    ch17 = dens_e_vp * es * r
    out_feats = np.concatenate([feats, ch17[:, None, :]], 1)  # [128,18,625]
    return out_feats.transpose(1, 0, 2).reshape(NCH, -1), d_tot.reshape(-1)


# ------------------------------------------------------------ device kernel
_CACHE = {}
LAST_RESULT = None  # BassKernelResults of the most recent device run


def _build_program(nbatch):
    import concourse.mybir as mybir
    from concourse import bacc, tile

    f32 = mybir.dt.float32
    bf16 = mybir.dt.bfloat16
    f16 = mybir.dt.float16
    K1HL = 2 * K1
    nbb = sum(nbatch)
    NV = SLAB * GRID[1] * GRID[2]  # 80000

    nc = bacc.Bacc("TRN2", target_bir_lowering=False, debug=True)
    lhs1_d = nc.dram_tensor("lhs1", [nbb, K1HL, 128], f16, kind="ExternalInput")
    lhs2_d = nc.dram_tensor("lhs2", [nbb, 128, NCH], bf16, kind="ExternalInput")
    rhs1_d = nc.dram_tensor("rhs1", [K1HL, WIN], f16, kind="ExternalInput")
    dens_e_d = nc.dram_tensor("dens_e", [128, VPP], f32, kind="ExternalInput")
    es_d = nc.dram_tensor("es", [128, 1], f32, kind="ExternalInput")
    feats_d = nc.dram_tensor("out_feats", [NCH, NV], f32, kind="ExternalOutput")
    dens_d = nc.dram_tensor("out_dens", [NV], f32, kind="ExternalOutput")
    scratch_d = nc.dram_tensor("scratch", [NCH, NV], f32)

    with tile.TileContext(nc) as tc:
        HALF = WIN // 2  # 1400
        with (
            tc.tile_pool(name="const", bufs=1) as constp,
            tc.tile_pool(name="w1", bufs=4) as w1p,
            tc.tile_pool(name="w2", bufs=4) as w2p,
            tc.tile_pool(name="dens", bufs=4) as densp,
            tc.tile_pool(name="stage", bufs=2) as stagep,
            tc.tile_pool(name="pm", bufs=2, space="PSUM") as pmp,
            tc.tile_pool(name="po", bufs=2, space="PSUM") as pop,
        ):
            rhs1_t = constp.tile([K1HL, WIN], f16)
            nc.sync.dma_start(out=rhs1_t[:], in_=rhs1_d[:])

            # flat list of (bin, batch, half, chunk) work units
            units = []
            bb = 0
            for b in range(NBINS):
                for h0 in (0, HALF):
                    for c0, cw in _chunks(HALF, CHUNK):
                        for t in range(nbatch[b]):
                            units.append((b, bb + t, t, h0, c0, cw))
                bb += nbatch[b]

            lhs_tiles = {}   # batch index -> (l1, l2)
            po_tiles = {}    # (bin, half) -> po tile
            stage_tiles = {}  # bin -> stage tile
            pm_tiles = {}    # unit i -> (pm tile, unit)

            def load_lhs(bi):
                if bi in lhs_tiles:
                    return
                l1 = w1p.tile([K1HL, 128], f16, tag="l1")
                nc.gpsimd.dma_start(out=l1[:], in_=lhs1_d[bi])
                l2 = w2p.tile([128, NCH], bf16, tag="l2")
                nc.gpsimd.dma_start(out=l2[:], in_=lhs2_d[bi])
                lhs_tiles[bi] = (l1, l2)

            def emit_m1(i):
                b, bi, t, h0, c0, cw = units[i]
                load_lhs(bi)
                pm_t = pmp.tile([128, CHUNK], f32, tag="pm")
                nc.tensor.matmul(
                    out=pm_t[:, :cw],
                    lhsT=lhs_tiles[bi][0][:],
                    rhs=rhs1_t[:, h0 + c0:h0 + c0 + cw],
                    start=True,
                    stop=True,
                )
                pm_tiles[i] = pm_t

            def emit_tail(i):
                b, bi, t, h0, c0, cw = units[i]
                pm_t = pm_tiles.pop(i)
                d_t = densp.tile([128, CHUNK], bf16, tag="d")
                nc.scalar.activation(
                    out=d_t[:, :cw],
                    in_=pm_t[:, :cw],
                    func=mybir.ActivationFunctionType.Exp,
                    scale=-0.5,
                )
                if (b, h0) not in po_tiles:
                    po_tiles[(b, h0)] = pop.tile([NCH, HALF], f32, tag="po", name="po_t")
                nc.tensor.matmul(
                    out=po_tiles[(b, h0)][:, c0:c0 + cw],
                    lhsT=lhs_tiles[bi][1][:],
                    rhs=d_t[:, :cw],
                    start=(t == 0),
                    stop=(t == nbatch[b] - 1),
                )
                # when this was the unit finishing a half: drain it
                last_in_half = (t == nbatch[b] - 1) and (c0 + cw == HALF)
                if last_in_half:
                    if b not in stage_tiles:
                        stage_tiles[b] = stagep.tile(
                            [NCH, WIN], f32, tag="stage", name="stage_t"
                        )
                    nc.vector.tensor_copy(
                        out=stage_tiles[b][:, h0:h0 + HALF],
                        in_=po_tiles.pop((b, h0))[:],
                    )
                    if h0 == HALF:  # bin complete -> write scratch
                        jw = min(BJ, GRID[1] - BJ * b)
                        w = jw * SLAB * GRID[2]
                        off = b * BJ * SLAB * GRID[2]
                        nc.sync.dma_start(
                            out=scratch_d[:, off:off + w],
                            in_=stage_tiles.pop(b)[:, :w],
                        )

            SKEW = 1
            n = len(units)
            for i in range(n + SKEW):
                if i < n:
                    emit_m1(i)
                if i >= SKEW:
                    emit_tail(i - SKEW)

        # ---- phase 2: repartition + empty term + normalize ----
        with (
            tc.tile_pool(name="fin", bufs=1) as finp,
            tc.tile_pool(name="aux", bufs=1) as auxp,
        ):
            fin_t = finp.tile([128, NCH, VPP], f32)
            nc.sync.dma_start(
                out=fin_t[:],
                in_=scratch_d[:].rearrange("c (p r) -> p c r", p=128),
            )
            de_t = auxp.tile([128, VPP], f32)
            nc.sync.dma_start(out=de_t[:], in_=dens_e_d[:])
            es_t = auxp.tile([128, 1], f32)
            nc.sync.dma_start(out=es_t[:], in_=es_d[:])

            dtot_t = auxp.tile([128, VPP], f32)
            nc.vector.tensor_add(out=dtot_t[:], in0=fin_t[:, 17], in1=de_t[:])
            r_t = auxp.tile([128, VPP], f32)
            nc.vector.reciprocal(out=r_t[:], in_=dtot_t[:])
            for c in range(17):
                nc.vector.tensor_mul(
                    out=fin_t[:, c], in0=fin_t[:, c], in1=r_t[:]
                )
            # ch17 = es * dens_e * r
            nc.vector.tensor_mul(out=de_t[:], in0=de_t[:], in1=r_t[:])
            nc.vector.tensor_scalar_mul(
                out=fin_t[:, 17], in0=de_t[:], scalar1=es_t[:]
            )
            nc.sync.dma_start(
                out=feats_d[:].rearrange("c (p r) -> p c r", p=128),
                in_=fin_t[:],
            )
            nc.sync.dma_start(
                out=dens_d[:].rearrange("(p r) -> p r", p=128), in_=dtot_t[:]
            )
    if not nc.is_finalized():
        nc.finalize()
    return nc


def _hilo_fp16(lhs1):
    """Split fp32 tables into stacked fp16 hi/lo rows (one bf16-rate matmul)."""
    hi = lhs1.astype(np.float16)
    lo = (lhs1 - hi.astype(np.float32)).astype(np.float16)
    return np.concatenate([hi, lo], axis=-2)  # [..., 2*K1, 128]


def kernel(means3d, opacities, features, scales, rotations, empty_scalar):
    lhs1, lhs2, nbatch = _host_prep(
        means3d, opacities, features, scales, rotations
    )
    rhs1 = _basis_rhs()
    dens_e_cores = _dens_e_vprime()
    es = np.float32(np.asarray(empty_scalar).reshape(-1)[0])

    H, W, D = GRID
    rhs1_hl = np.concatenate(
        [rhs1.astype(np.float16), rhs1.astype(np.float16)], axis=0
    )  # [2*K1, WIN]

    if os.environ.get("KERNEL_GOLDEN"):
        feats_parts, dens_parts = [], []
        for s in range(NCORES):
            f, d = _golden_core(
                _hilo_fp16(lhs1[s]), lhs2[s].astype(ml_dtypes.bfloat16),
                nbatch, rhs1_hl, dens_e_cores[s], es,
            )
            feats_parts.append(f)
            dens_parts.append(d)
    else:
        from concourse.bass_utils import run_bass_kernel_spmd

        key = tuple(nbatch)
        if key not in _CACHE:
            _CACHE[key] = _build_program(nbatch)
        nc = _CACHE[key]
        in_maps = [
            {
                "lhs1": np.ascontiguousarray(_hilo_fp16(lhs1[s])),
                "lhs2": np.ascontiguousarray(lhs2[s]).astype(ml_dtypes.bfloat16),
                "rhs1": rhs1_hl,
                "dens_e": dens_e_cores[s],
                "es": np.full((128, 1), es, np.float32),
            }
            for s in range(NCORES)
        ]
        res = run_bass_kernel_spmd(
            nc,
            in_maps,
            list(range(NCORES)),
            trace=bool(os.environ.get("KERNEL_TRACE")),
        )
        global LAST_RESULT
        LAST_RESULT = res
        feats_parts = [res.results[s]["out_feats"] for s in range(NCORES)]
        dens_parts = [res.results[s]["out_dens"] for s in range(NCORES)]

    feats = np.concatenate(
        [
            f.reshape(NCH, W, SLAB, D).transpose(2, 1, 3, 0)
            for f in feats_parts
        ],
        axis=0,
    )  # [200,200,16,18]
    dens = np.concatenate(
        [d.reshape(W, SLAB, D).transpose(1, 0, 2) for d in dens_parts], axis=0
    )[..., None]  # [200,200,16,1]
    return dens.astype(np.float32), feats.astype(np.float32)


# revision 29
# speedup vs baseline: 1.3651x; 1.0779x over previous
"""GaussianVoxelizer Trainium2 kernel (8 NeuronCores).

Strategy: the 200x200x16 output grid is sharded over 8 cores as 25-row
I-slabs.  On the host, each gaussian is routed (with duplication) to the
I-slabs and 7-wide J-bins its 3-sigma box overlaps.  Per (slab, J-bin)
the device evaluates, for a batch of <=128 gaussians, the Mahalanobis
quadratic over the bin's 25x7x16 voxel window with ONE fp32 matmul
(one-hot axis tables + centered cross monomials as the stationary
basis; box masks enter as additive barriers, opacity as -2*ln(op)),
takes exp on the scalar engine, and contracts gaussians against the
18 feature channels (17 feats + 1 density row) with a bf16 matmul
accumulated in PSUM.  A second device phase repartitions the raw sums
to 128 partitions, adds the (constant, separable) "empty gaussian"
density, and performs the normalization divide.  Host only routes
inputs and reassembles the sharded outputs.
"""

import os
import sys
import numpy as np

for _p in ("/opt/trn_rl_repo", "/opt/pypackages"):
    if os.path.isdir(_p) and _p not in sys.path:
        sys.path.append(_p)

import ml_dtypes

# ---------------------------------------------------------------- constants
VOL_LO = np.array([-40.0, -40.0, -1.0], np.float32)
VOL_HI = np.array([40.0, 40.0, 5.4], np.float32)
VOXEL = np.float32(0.4)
GRID = (200, 200, 16)
NCORES = 8
SLAB = 25            # I-rows per core
BJ = 7               # J-bin width
NBINS = (GRID[1] + BJ - 1) // BJ       # 29 (last bin is 4 wide)
WIN = SLAB * BJ * GRID[2]              # 2800 voxels per window
K1 = SLAB + BJ + GRID[2] + 3           # 51 basis rows for the maha matmul
CI, CJ, CK = 12.0, 3.0, 7.5            # fixed centering of cross monomials
BARRIER = 4000.0
NCH = 18                               # 17 feature channels + density row
CHUNK = 512
VPP = (SLAB * GRID[1] * GRID[2]) // 128  # 625 voxels per partition in phase 2


def _chunks(total, step):
    return [(c, min(step, total - c)) for c in range(0, total, step)]


# ---------------------------------------------------------------- host prep
def _quat_to_rotmat(q):
    q = q / np.linalg.norm(q, axis=-1, keepdims=True)
    w, x, y, z = q[..., 0], q[..., 1], q[..., 2], q[..., 3]
    R = np.stack(
        [
            1 - 2 * (y * y + z * z), 2 * (x * y - w * z), 2 * (x * z + w * y),
            2 * (x * y + w * z), 1 - 2 * (x * x + z * z), 2 * (y * z - w * x),
            2 * (x * z - w * y), 2 * (y * z + w * x), 1 - 2 * (x * x + y * y),
        ],
        axis=-1,
    ).astype(np.float32)
    return R.reshape(q.shape[:-1] + (3, 3))


def _host_prep(means3d, opacities, features, scales, rotations):
    """Mirror of the reference's per-gaussian math (numpy f32) + routing.

    Returns per-core lhs tables and the static (bin -> nbatch) schedule.
    """
    N = means3d.shape[0]
    lo, hi = VOL_LO, VOL_HI

    R = _quat_to_rotmat(rotations.astype(np.float32))
    Lm = R * scales.astype(np.float32)[:, None, :]
    cov = (Lm @ np.swapaxes(Lm, -1, -2)).astype(np.float32)
    cov_inv = np.linalg.inv(cov).astype(np.float32)
    sig = np.sqrt(np.diagonal(cov, axis1=-2, axis2=-1)).astype(np.float32)

    b_lo = np.clip(means3d - np.float32(3.0) * sig, lo, hi).astype(np.float32)
    b_hi = np.clip(means3d + np.float32(3.0) * sig, lo, hi).astype(np.float32)
    valid = (((b_lo > lo) | (b_hi > lo)) & ((b_lo < hi) | (b_hi < hi))).all(-1)
    i_lo = ((b_lo - lo) / VOXEL).astype(np.int32)
    i_hi = ((b_hi - lo) / VOXEL).astype(np.int32)

    # inclusive, grid-clamped box
    box_lo = np.maximum(i_lo, 0)
    box_hi = np.minimum(i_hi, np.array(GRID, np.int32) - 1)
    valid &= (box_lo <= box_hi).all(-1)

    Ct = (VOXEL * VOXEL) * cov_inv          # quadratic form in voxel coords
    m_t = (means3d - lo) / VOXEL - np.float32(0.5)  # voxel-coord mean
    op = np.maximum(opacities[:, 0].astype(np.float32), np.float32(1e-30))
    lnop = np.log(op).astype(np.float32)

    # ---- routing: entries[(core, bin)] = list of gaussian ids
    entries = [[[] for _ in range(NBINS)] for _ in range(NCORES)]
    idxs = np.nonzero(valid)[0]
    for g in idxs:
        s0 = box_lo[g, 0] // SLAB
        s1 = box_hi[g, 0] // SLAB
        b0 = box_lo[g, 1] // BJ
        b1 = box_hi[g, 1] // BJ
        for s in range(s0, s1 + 1):
            for b in range(b0, b1 + 1):
                entries[s][b].append(g)

    # static schedule: nbatch per bin = max over cores
    nbatch = [
        max(1, max((len(entries[s][b]) + 127) // 128 for s in range(NCORES)))
        for b in range(NBINS)
    ]
    nbb = sum(nbatch)

    il = np.arange(SLAB, dtype=np.float32)
    jl = np.arange(BJ, dtype=np.float32)
    kk = np.arange(GRID[2], dtype=np.float32)

    lhs1 = np.zeros((NCORES, nbb, K1, 128), np.float32)
    lhs1[:, :, 0:SLAB, :] = BARRIER          # pad gaussians -> dens 0
    lhs2 = np.zeros((NCORES, nbb, 128, NCH), np.float32)

    for s in range(NCORES):
        bb = 0
        for b in range(NBINS):
            glist = entries[s][b]
            for t in range(nbatch[b]):
                part = glist[t * 128:(t + 1) * 128]
                if part:
                    gs = np.asarray(part)
                    n = len(gs)
                    aI = m_t[gs, 0] - np.float32(SLAB * s)
                    aJ = m_t[gs, 1] - np.float32(BJ * b)
                    ak = m_t[gs, 2]
                    c00 = Ct[gs, 0, 0]; c11 = Ct[gs, 1, 1]; c22 = Ct[gs, 2, 2]
                    c01 = Ct[gs, 0, 1]; c02 = Ct[gs, 0, 2]; c12 = Ct[gs, 1, 2]

                    LI = 2 * c01 * (CJ - aJ) + 2 * c02 * (CK - ak)
                    LJ = 2 * c01 * (CI - aI) + 2 * c12 * (CK - ak)
                    Lk = 2 * c02 * (CI - aI) + 2 * c12 * (CJ - aJ)
                    CONST = (
                        2 * (c01 * (CI - aI) * (CJ - aJ)
                             + c02 * (CI - aI) * (CK - ak)
                             + c12 * (CJ - aJ) * (CK - ak))
                        - 2 * lnop[gs]
                    )

                    tabI = (c00[:, None] * (il[None] - aI[:, None]) ** 2
                            + LI[:, None] * (il[None] - CI))
                    tabJ = (c11[:, None] * (jl[None] - aJ[:, None]) ** 2
                            + LJ[:, None] * (jl[None] - CJ))
                    tabk = (c22[:, None] * (kk[None] - ak[:, None]) ** 2
                            + Lk[:, None] * (kk[None] - CK) + CONST[:, None])

                    # box-mask barriers (global coords)
                    gi = SLAB * s + np.arange(SLAB)[None]
                    gj = BJ * b + np.arange(BJ)[None]
                    gk = np.arange(GRID[2])[None]
                    tabI = np.where((gi < box_lo[gs, 0:1]) | (gi > box_hi[gs, 0:1]),
                                    np.float32(BARRIER), tabI)
                    tabJ = np.where((gj < box_lo[gs, 1:2]) | (gj > box_hi[gs, 1:2]),
                                    np.float32(BARRIER), tabJ)
                    tabk = np.where((gk < box_lo[gs, 2:3]) | (gk > box_hi[gs, 2:3]),
                                    np.float32(BARRIER), tabk)

                    col = np.concatenate(
                        [tabI, tabJ, tabk,
                         2 * c01[:, None], 2 * c02[:, None], 2 * c12[:, None]],
                        axis=1,
                    )  # [n, K1]
                    lhs1[s, bb, :, :n] = col.T
                    lhs2[s, bb, :n, 0:17] = features[gs].astype(np.float32)
                    lhs2[s, bb, :n, 17] = 1.0
                bb += 1
    return lhs1, lhs2, nbatch


def _basis_rhs():
    """[K1, WIN] shared basis: one-hots + centered cross monomials.

    Window voxel order is J-major: v = jl*400 + il*16 + k.
    """
    il, jl, kk = np.meshgrid(
        np.arange(SLAB), np.arange(BJ), np.arange(GRID[2]), indexing="ij"
    )
    v = jl * (SLAB * GRID[2]) + il * GRID[2] + kk
    rhs = np.zeros((K1, WIN), np.float32)
    fil, fjl, fkk = (x.reshape(-1) for x in (il, jl, kk))
    fv = v.reshape(-1)
    for r in range(SLAB):
        rhs[r, fv[fil == r]] = 1.0
    for r in range(BJ):
        rhs[SLAB + r, fv[fjl == r]] = 1.0
    for r in range(GRID[2]):
        rhs[SLAB + BJ + r, fv[fkk == r]] = 1.0
    rhs[K1 - 3, fv] = (fil - CI) * (fjl - CJ)
    rhs[K1 - 2, fv] = (fil - CI) * (fkk - CK)
    rhs[K1 - 1, fv] = (fjl - CJ) * (fkk - CK)
    return rhs


def _bin_groups():
    """Phase-2 overlap groups: [(first_bin, nbins, v_base, r_base, r_len)].

    Each group's j-columns are a multiple of 8 so its voxels split evenly
    over 128 partitions.
    """
    groups = []
    v0 = r0 = 0
    for b0 in range(0, NBINS, 8):
        nb = min(8, NBINS - b0)
        jcnt = sum(min(BJ, GRID[1] - BJ * b) for b in range(b0, b0 + nb))
        size = jcnt * SLAB * GRID[2]
        rlen = size // 128
        groups.append((b0, nb, v0, r0, rlen))
        v0 += size
        r0 += rlen
    assert r0 == VPP and v0 == SLAB * GRID[1] * GRID[2]
    return groups


def _dens_e_vprime():
    """Empty-gaussian density per core, flat in v' = j*400 + i*16 + k."""
    H, W, D = GRID
    ii, jj, kk = np.meshgrid(
        np.arange(H, dtype=np.float32),
        np.arange(W, dtype=np.float32),
        np.arange(D, dtype=np.float32),
        indexing="ij",
    )
    gc = np.stack([ii, jj, kk], -1) + np.float32(0.5)
    gc = gc * VOXEL + VOL_LO
    e_mean = (VOL_LO + VOL_HI) / 2
    e_rng = VOL_HI - VOL_LO
    de = (gc - e_mean) / e_rng
    maha = (de * de).sum(-1)
    dens_e = np.exp(np.float32(-0.5) * maha).astype(np.float32)  # [200,200,16]
    out = []
    for s in range(NCORES):
        slab = dens_e[SLAB * s:SLAB * (s + 1)]          # [25,200,16]
        out.append(slab.transpose(1, 0, 2).reshape(-1).copy())  # v'-flat
    return out


def _dens_e_device_layout(de_flat):
    """v'-flat [80000] -> device fin layout [128, VPP] (grouped)."""
    out = np.empty((128, VPP), np.float32)
    for b0, nb, v0, r0, rlen in _bin_groups():
        blk = de_flat[v0:v0 + rlen * 128].reshape(128, rlen)
        out[:, r0:r0 + rlen] = blk
    return out


# ------------------------------------------------------------- golden model
def _golden_core(lhs1, lhs2, nbatch, rhs1, dens_e_vp, es):
    """Numpy simulation of the device program for one core.

    lhs1: [nbb, 2*K1, 128] fp16 hi/lo stacked; rhs1: [2*K1, WIN] fp16.
    """
    scratch = np.zeros((NCH, SLAB * GRID[1] * GRID[2]), np.float32)
    bb = 0
    for b in range(NBINS):
        jw = min(BJ, GRID[1] - BJ * b)
        acc = np.zeros((NCH, WIN), np.float32)
        for t in range(nbatch[b]):
            maha = lhs1[bb].T.astype(np.float32) @ rhs1.astype(np.float32)
            dens = np.exp(-0.5 * maha).astype(ml_dtypes.bfloat16)
            acc += lhs2[bb].astype(ml_dtypes.bfloat16).T.astype(np.float32) @ \
                dens.astype(np.float32)
            bb += 1
        w = jw * SLAB * GRID[2]
        scratch[:, b * BJ * SLAB * GRID[2]:][:, :w] = acc[:, :w]
    d_tot = scratch[17] + dens_e_vp
    r = (1.0 / d_tot).astype(np.float32)
    feats = scratch[0:17] * r[None]
    ch17 = dens_e_vp * es * r
    out_feats = np.concatenate([feats, ch17[None]], 0)  # [18, 80000]
    return out_feats, d_tot


# ------------------------------------------------------------ device kernel
_CACHE = {}
LAST_RESULT = None  # BassKernelResults of the most recent device run


def _build_program(nbatch):
    import concourse.mybir as mybir
    from concourse import bacc, tile

    f32 = mybir.dt.float32
    bf16 = mybir.dt.bfloat16
    f16 = mybir.dt.float16
    K1HL = 2 * K1
    nbb = sum(nbatch)
    NV = SLAB * GRID[1] * GRID[2]  # 80000

    nc = bacc.Bacc("TRN2", target_bir_lowering=False, debug=True)
    lhs1_d = nc.dram_tensor("lhs1", [nbb, K1HL, 128], f16, kind="ExternalInput")
    lhs2_d = nc.dram_tensor("lhs2", [nbb, 128, NCH], bf16, kind="ExternalInput")
    rhs1_d = nc.dram_tensor("rhs1", [K1HL, WIN], f16, kind="ExternalInput")
    dens_e_d = nc.dram_tensor("dens_e", [128, VPP], f32, kind="ExternalInput")
    es_d = nc.dram_tensor("es", [128, 1], f32, kind="ExternalInput")
    feats_d = nc.dram_tensor("out_feats", [NCH, NV], f32, kind="ExternalOutput")
    dens_d = nc.dram_tensor("out_dens", [NV], f32, kind="ExternalOutput")
    scratch_d = nc.dram_tensor("scratch", [NCH, NV], f32)

    with tile.TileContext(nc) as tc:
        HALF = WIN // 2  # 1400
        groups = _bin_groups()
        with (
            tc.tile_pool(name="const", bufs=1) as constp,
            tc.tile_pool(name="w1", bufs=4) as w1p,
            tc.tile_pool(name="w2", bufs=4) as w2p,
            tc.tile_pool(name="dens", bufs=4) as densp,
            tc.tile_pool(name="stage", bufs=2) as stagep,
            tc.tile_pool(name="fin", bufs=1) as finp,
            tc.tile_pool(name="aux", bufs=1) as auxp,
            tc.tile_pool(name="pm", bufs=2, space="PSUM") as pmp,
            tc.tile_pool(name="po", bufs=2, space="PSUM") as pop,
        ):
            rhs1_t = constp.tile([K1HL, WIN], f16)
            for r0, rw in _chunks(WIN, HALF):
                nc.scalar.dma_start(
                    out=rhs1_t[:, r0:r0 + rw], in_=rhs1_d[:, r0:r0 + rw]
                )

            fin_t = finp.tile([128, NCH, VPP], f32)
            de_t = auxp.tile([128, VPP], f32)
            nc.scalar.dma_start(out=de_t[:], in_=dens_e_d[:])
            es_t = auxp.tile([128, 1], f32)
            nc.scalar.dma_start(out=es_t[:], in_=es_d[:])
            dtot_t = auxp.tile([128, VPP], f32)
            r_t = auxp.tile([128, VPP], f32)

            def emit_phase2_group(gi):
                b0, nb, v0, r0, rlen = groups[gi]
                size = rlen * 128
                sl = slice(r0, r0 + rlen)
                nc.sync.dma_start(
                    out=fin_t[:, :, sl],
                    in_=scratch_d[:, v0:v0 + size].rearrange(
                        "c (p r) -> p c r", p=128
                    ),
                )
                nc.vector.tensor_add(
                    out=dtot_t[:, sl], in0=fin_t[:, 17, sl], in1=de_t[:, sl]
                )
                nc.vector.reciprocal(out=r_t[:, sl], in_=dtot_t[:, sl])
                for c in range(17):
                    nc.vector.tensor_mul(
                        out=fin_t[:, c, sl], in0=fin_t[:, c, sl],
                        in1=r_t[:, sl],
                    )
                # ch17 = es * dens_e * r
                nc.vector.tensor_mul(
                    out=r_t[:, sl], in0=r_t[:, sl], in1=de_t[:, sl]
                )
                nc.vector.tensor_scalar_mul(
                    out=fin_t[:, 17, sl], in0=r_t[:, sl], scalar1=es_t[:]
                )
                nc.sync.dma_start(
                    out=feats_d[:, v0:v0 + size].rearrange(
                        "c (p r) -> p c r", p=128
                    ),
                    in_=fin_t[:, :, sl],
                )
                nc.sync.dma_start(
                    out=dens_d[v0:v0 + size].rearrange("(p r) -> p r", p=128),
                    in_=dtot_t[:, sl],
                )

            # flat list of (bin, batch, half, chunk) work units
            units = []
            bb = 0
            for b in range(NBINS):
                for h0 in (0, HALF):
                    for c0, cw in _chunks(HALF, CHUNK):
                        for t in range(nbatch[b]):
                            units.append((b, bb + t, t, h0, c0, cw))
                bb += nbatch[b]

            lhs_tiles = {}   # batch index -> (l1, l2)
            po_tiles = {}    # (bin, half) -> po tile
            stage_tiles = {}  # bin -> stage tile
            pm_tiles = {}    # unit i -> (pm tile, unit)

            def load_lhs(bi):
                if bi in lhs_tiles:
                    return
                l1 = w1p.tile([K1HL, 128], f16, tag="l1")
                nc.gpsimd.dma_start(out=l1[:], in_=lhs1_d[bi])
                l2 = w2p.tile([128, NCH], bf16, tag="l2")
                nc.gpsimd.dma_start(out=l2[:], in_=lhs2_d[bi])
                lhs_tiles[bi] = (l1, l2)

            def emit_m1(i):
                b, bi, t, h0, c0, cw = units[i]
                load_lhs(bi)
                pm_t = pmp.tile([128, CHUNK], f32, tag="pm")
                nc.tensor.matmul(
                    out=pm_t[:, :cw],
                    lhsT=lhs_tiles[bi][0][:],
                    rhs=rhs1_t[:, h0 + c0:h0 + c0 + cw],
                    start=True,
                    stop=True,
                )
                pm_tiles[i] = pm_t

            def emit_tail(i):
                b, bi, t, h0, c0, cw = units[i]
                pm_t = pm_tiles.pop(i)
                d_t = densp.tile([128, CHUNK], bf16, tag="d")
                nc.scalar.activation(
                    out=d_t[:, :cw],
                    in_=pm_t[:, :cw],
                    func=mybir.ActivationFunctionType.Exp,
                    scale=-0.5,
                )
                if (b, h0) not in po_tiles:
                    po_tiles[(b, h0)] = pop.tile([NCH, HALF], f32, tag="po", name="po_t")
                nc.tensor.matmul(
                    out=po_tiles[(b, h0)][:, c0:c0 + cw],
                    lhsT=lhs_tiles[bi][1][:],
                    rhs=d_t[:, :cw],
                    start=(t == 0),
                    stop=(t == nbatch[b] - 1),
                )
                # when this was the unit finishing a half: drain it
                last_in_half = (t == nbatch[b] - 1) and (c0 + cw == HALF)
                if last_in_half:
                    if b not in stage_tiles:
                        stage_tiles[b] = stagep.tile(
                            [NCH, WIN], f32, tag="stage", name="stage_t"
                        )
                    nc.vector.tensor_copy(
                        out=stage_tiles[b][:, h0:h0 + HALF],
                        in_=po_tiles.pop((b, h0))[:],
                    )
                    if h0 == HALF:  # bin complete -> write scratch
                        jw = min(BJ, GRID[1] - BJ * b)
                        w = jw * SLAB * GRID[2]
                        off = b * BJ * SLAB * GRID[2]
                        nc.sync.dma_start(
                            out=scratch_d[:, off:off + w],
                            in_=stage_tiles.pop(b)[:, :w],
                        )
                        for gi, (b0, nb, _, _, _) in enumerate(groups):
                            if b == b0 + nb - 1:
                                emit_phase2_group(gi)

            SKEW = 1
            n = len(units)
            for i in range(n + SKEW):
                if i < n:
                    emit_m1(i)
                if i >= SKEW:
                    emit_tail(i - SKEW)
    if not nc.is_finalized():
        nc.finalize()
    return nc


def _hilo_fp16(lhs1):
    """Split fp32 tables into stacked fp16 hi/lo rows (one bf16-rate matmul)."""
    hi = lhs1.astype(np.float16)
    lo = (lhs1 - hi.astype(np.float32)).astype(np.float16)
    return np.concatenate([hi, lo], axis=-2)  # [..., 2*K1, 128]


def kernel(means3d, opacities, features, scales, rotations, empty_scalar):
    lhs1, lhs2, nbatch = _host_prep(
        means3d, opacities, features, scales, rotations
    )
    rhs1 = _basis_rhs()
    dens_e_cores = _dens_e_vprime()
    es = np.float32(np.asarray(empty_scalar).reshape(-1)[0])

    H, W, D = GRID
    rhs1_hl = np.concatenate(
        [rhs1.astype(np.float16), rhs1.astype(np.float16)], axis=0
    )  # [2*K1, WIN]

    if os.environ.get("KERNEL_GOLDEN"):
        feats_parts, dens_parts = [], []
        for s in range(NCORES):
            f, d = _golden_core(
                _hilo_fp16(lhs1[s]), lhs2[s].astype(ml_dtypes.bfloat16),
                nbatch, rhs1_hl, dens_e_cores[s], es,
            )
            feats_parts.append(f)
            dens_parts.append(d)
    else:
        from concourse.bass_utils import run_bass_kernel_spmd

        key = tuple(nbatch)
        if key not in _CACHE:
            _CACHE[key] = _build_program(nbatch)
        nc = _CACHE[key]
        in_maps = [
            {
                "lhs1": np.ascontiguousarray(_hilo_fp16(lhs1[s])),
                "lhs2": np.ascontiguousarray(lhs2[s]).astype(ml_dtypes.bfloat16),
                "rhs1": rhs1_hl,
                "dens_e": _dens_e_device_layout(dens_e_cores[s]),
                "es": np.full((128, 1), es, np.float32),
            }
            for s in range(NCORES)
        ]
        res = run_bass_kernel_spmd(
            nc,
            in_maps,
            list(range(NCORES)),
            trace=bool(os.environ.get("KERNEL_TRACE")),
        )
        global LAST_RESULT
        LAST_RESULT = res
        feats_parts = [res.results[s]["out_feats"] for s in range(NCORES)]
        dens_parts = [res.results[s]["out_dens"] for s in range(NCORES)]

    feats = np.concatenate(
        [
            f.reshape(NCH, W, SLAB, D).transpose(2, 1, 3, 0)
            for f in feats_parts
        ],
        axis=0,
    )  # [200,200,16,18]
    dens = np.concatenate(
        [d.reshape(W, SLAB, D).transpose(1, 0, 2) for d in dens_parts], axis=0
    )[..., None]  # [200,200,16,1]
    return dens.astype(np.float32), feats.astype(np.float32)
